# revision 32
# baseline (speedup 1.0000x reference)
"""Trainium2 Bass kernel for nn_MaxPooling (segment_max pooling + max-node
attention scores).

Strategy (per the segment-aligned sharding hint):
  - 1024 graphs are split 128-per-core across 8 NeuronCores; batch is sorted,
    so each graph's nodes are a contiguous row range of x.
  - Host packs each graph transposed: x_pad[core] is [128(hidden), S_total]
    with graph slot j occupying columns [off_j, off_j + S_j), padded with
    -inf to a per-slot size S_j (max over the 8 cores, 16-aligned).  With
    hidden on partitions:
      * segment_max  = one free-axis reduce_max per graph        (DVE)
      * per-dim match = tensor_scalar is_equal vs the gmax column (DVE/GPSIMD)
        or Sign(x - gmax) on the Scalar engine (ACT)  -> bf16 tile
      * per-node match count = ones^T @ match  (TensorE matmul, PSUM row per
        graph)  -> counts[graph, node]
      * scores = counts >= thr (thr 0.5 for eq rows, -127.5 for sign rows,
        PSUM pad memset to -1e9 so pad columns score 0)
      * totals = row-sum, attention = scores * (1/totals)
  - Outputs: gmax as [hidden, 128] per core and attention as [128, PADW]
    per core; host transposes/trims back into the full outputs.

Pad-node correctness: pad columns are -inf; for a non-empty graph the gmax
column is finite so is_equal(-inf, gmax)=0 and Sign(-inf - gmax)=-1, i.e.
pad nodes never score.  Empty graphs (cannot occur for these inputs) would
only corrupt their own discarded rows; their gmax stays -inf which matches
jax.ops.segment_max's identity.
"""

import os
import numpy as np

H = 128          # hidden dim == SBUF partitions
N_CORES = 8

# engine split for the per-graph match pass: slots [0, N_ACT) on ScalarE
# (Sign path), the rest on VectorE (is_equal).  GPSIMD is not used: its
# tensor-scalar is ~16 us per graph and it contends for the DVE SBUF port.
N_ACT = int(os.environ.get("K_NA", "128"))


def _eng_split(g_loc):
    return ["a" if j < N_ACT else "v" for j in range(g_loc)]

GROUP_W = 2560   # columns per input DMA (~1.25 MiB per dma_start)

_last_results = None  # BassKernelResults from the most recent run (for test.py)


def _apply_tile_patch():
    """This walrus build rejects instructions with >2 sync waits; the Tile
    kernel-tail drain accumulates one wait per live semaphore.  Split the
    drain's waits across single-wait SP nops."""
    import concourse.mybir as mybir
    import concourse.tile as tile_mod
    from concourse.vector_clock import ScopedClock

    if getattr(tile_mod.TileContext, "_ant_drain_patched", False):
        return

    def _drain_and_barrier(self, tick_clock, wait_clock):
        nc = self.nc
        drain_inst = nc.sync.drain()
        wait_clock.add_sem_waits(
            drain_inst.ins, ScopedClock({None: tick_clock.global_clock})
        )
        si = drain_inst.ins.sync_info
        waits = list(si.on_wait or []) if si is not None else []
        if len(waits) > 1:
            si.on_wait = waits[:1]
            for w in waits[1:]:
                nop = nc.sync.nop()
                nop.ins.sync_info = mybir.SyncInfo(on_wait=[w], on_update=[])
        nc.all_engine_barrier()
        assert self.sems is not None
        popped = nc._tile_sem_poison_stack.pop()
        assert popped is self._sem_poison
        nc.clear_and_free_semaphores(list(self.sems.allocated().values()))
        nc.all_engine_barrier()

    tile_mod.TileContext._drain_and_barrier = _drain_and_barrier
    tile_mod.TileContext._ant_drain_patched = True


def _split_excess_waits(nc, maxw=1):
    """Walrus here rejects instructions with more than ~1-2 sync waits.
    Hoist excess waits onto same-engine NoOps inserted just before the
    offending instruction (the engine blocks on the nop's wait first, so
    ordering semantics are identical)."""
    import bass_rust
    import concourse.mybir as mybir

    n = 0
    for f in nc.m.functions:
        for bb in f.blocks:
            out = []
            for inst in bb.instructions:
                si = inst.sync_info
                waits = list(si.on_wait or []) if si is not None else []
                if len(waits) > maxw and inst.engine is not None:
                    for i in range(0, len(waits) - maxw, maxw):
                        nop = bass_rust.InstNoOp(name=f"WSPLIT-{n}")
                        n += 1
                        nop.engine = inst.engine
                        nop.sync_info = mybir.SyncInfo(
                            on_wait=waits[i : i + maxw], on_update=[]
                        )
                        out.append(nop)
                    si.on_wait = waits[len(waits) - maxw :]
                out.append(inst)
            bb.instructions = out


def _install_ntff_hook_shim():
    """bass_utils hard-imports antenv.axon_hooks when trace=True under axon;
    this image's antenv lacks that module.  Provide it, wired to the
    libaxon_pjrt ctypes profiler from trn_agent_boot when available."""
    import sys
    import types

    try:
        import antenv.axon_hooks  # noqa: F401

        return
    except ImportError:
        pass
    try:
        import antenv
    except ImportError:
        return
    mod = types.ModuleType("antenv.axon_hooks")
    mod._hook = None
    mod.set_axon_ntff_profile_hook = lambda h: setattr(mod, "_hook", h)
    mod.get_axon_ntff_profile_hook = lambda: mod._hook
    sys.modules["antenv.axon_hooks"] = mod
    antenv.axon_hooks = mod
    try:
        from trn_agent_boot.trn_boot import _ntff_profile_via_ctypes

        hook = _ntff_profile_via_ctypes("/opt/axon/libaxon_pjrt.so")
        if hook is not None:
            mod._hook = hook
    except Exception:
        pass


def _build_nc(slot_sizes, g_loc):
    import concourse.bass as bass
    import concourse.mybir as mybir
    from concourse.tile import TileContext

    _apply_tile_patch()

    offs = np.concatenate([[0], np.cumsum(slot_sizes)])
    s_total = int(offs[-1])
    padw = int(max(slot_sizes))
    f32 = mybir.dt.float32
    bf16 = mybir.dt.bfloat16
    Alu = mybir.AluOpType
    X = mybir.AxisListType.X

    eng = _eng_split(g_loc)

    nc = bass.Bass()
    x_d = nc.dram_tensor("x", [H, s_total], f32, kind="ExternalInput")
    # per-graph-row constants: col 0 = score threshold (0.5 for is_equal
    # rows, -127.5 for Sign rows), col 1 = pad-column score correction
    # subtracted from the row total (padw - S_j for Sign rows, else 0).
    cst_d = nc.dram_tensor("cst", [g_loc, 2], f32, kind="ExternalInput")
    att_d = nc.dram_tensor("att", [g_loc, padw], f32, kind="ExternalOutput")
    gmx_d = nc.dram_tensor("gmx", [H, g_loc], f32, kind="ExternalOutput")

    # group consecutive slots into ~GROUP_W-column DMAs; the first few
    # slots go in single-graph DMAs so compute starts sooner
    groups = []
    cur = []
    cur_w = 0
    for j in range(g_loc):
        w = int(slot_sizes[j])
        if cur and cur_w + w > GROUP_W:
            groups.append(cur)
            cur, cur_w = [], 0
        cur.append(j)
        cur_w += w
    if cur:
        groups.append(cur)

    with TileContext(nc) as tc:
        with (
            tc.tile_pool(name="xg", bufs=8) as xp,
            tc.tile_pool(name="eq", bufs=8) as eqp,
            tc.tile_pool(name="small", bufs=1) as sp,
            tc.tile_pool(name="psum", bufs=1, space="PSUM") as pp,
        ):
            # sliding one-hot: win[:, 128-j : 256-j] is all-zero except an
            # all-ones column at free position j -> matmul lhsT that routes a
            # column-sum into PSUM partition row j (engine APs cannot start
            # at arbitrary partitions, so rows are selected via lhsT instead).
            # winn carries -1 so Sign rows accumulate (match-count - 128).
            win = sp.tile([H, 2 * g_loc], bf16)
            nc.gpsimd.memset(win[:], 0.0)
            nc.gpsimd.memset(win[:, g_loc : g_loc + 1], 1.0)
            winn = sp.tile([H, 2 * g_loc], bf16)
            nc.gpsimd.memset(winn[:], 0.0)
            nc.gpsimd.memset(winn[:, g_loc : g_loc + 1], -1.0)
            cst = sp.tile([g_loc, 2], f32)
            nc.sync.dma_start(out=cst[:], in_=cst_d[:])
            gsb = sp.tile([H, g_loc], f32)
            # counts[graph, node] accumulated in PSUM; pad columns stay 0
            counts = pp.tile([g_loc, padw], f32)
            nc.vector.memset(counts[:], 0.0)

            for grp in groups:
                g0, g1 = grp[0], grp[-1]
                base = int(offs[g0])
                gw = int(offs[g1 + 1]) - base
                xt = xp.tile([H, GROUP_W], f32, tag="xg")
                nc.sync.dma_start(out=xt[:, :gw], in_=x_d[:, base : base + gw])
                for j in grp:
                    lo = int(offs[j]) - base
                    sj = int(slot_sizes[j])
                    xg = xt[:, lo : lo + sj]
                    gcol = gsb[:, j : j + 1]
                    nc.vector.reduce_max(out=gcol, in_=xg, axis=X)
                    eqt = eqp.tile([H, padw], bf16, tag="eq")
                    eq = eqt[:, :sj]
                    if eng[j] == "v":
                        # eq in {0,1}; count row = match count m
                        nc.vector.tensor_single_scalar(
                            out=eq, in_=xg, scalar=gcol, op=Alu.is_equal
                        )
                        wt = win
                    else:
                        # sign(gmax - x) in {0 match, +1 not}; with the -1
                        # lhsT the count row = m - 128, thresholded -127.5
                        nc.scalar.activation(
                            out=eq,
                            in_=xg,
                            func=mybir.ActivationFunctionType.Sign,
                            bias=gcol,
                            scale=-1.0,
                        )
                        wt = winn
                    lhsT = wt[:, g_loc - j : 2 * g_loc - j]
                    for cs in range(0, sj, 512):
                        cw = min(512, sj - cs)
                        nc.tensor.matmul(
                            out=counts[:, cs : cs + cw],
                            lhsT=lhsT,
                            rhs=eqt[:, cs : cs + cw],
                            start=False,
                            stop=True,
                            skip_group_check=True,
                        )

            scores = sp.tile([g_loc, padw], f32)
            nc.vector.tensor_single_scalar(
                out=scores[:], in_=counts[:], scalar=cst[:, 0:1], op=Alu.is_ge
            )
            tot = sp.tile([g_loc, 1], f32)
            nc.vector.reduce_sum(out=tot[:], in_=scores[:], axis=X)
            # remove the pad columns' bogus unit scores on Sign rows
            nc.vector.tensor_sub(out=tot[:], in0=tot[:], in1=cst[:, 1:2])
            inv = sp.tile([g_loc, 1], f32)
            nc.vector.reciprocal(inv[:], tot[:])
            att = sp.tile([g_loc, padw], f32)
            nc.vector.tensor_scalar_mul(out=att[:], in0=scores[:], scalar1=inv[:, 0:1])
            nc.sync.dma_start(out=att_d[:], in_=att[:])
            nc.sync.dma_start(out=gmx_d[:], in_=gsb[:])

    _split_excess_waits(nc)
    return nc


def kernel(x, batch, num_graphs):
    global _last_results
    x = np.ascontiguousarray(np.asarray(x, dtype=np.float32))
    batch = np.asarray(batch).astype(np.int64)
    num_graphs = int(num_graphs)
    n_nodes, hidden = x.shape
    assert hidden == H and num_graphs % N_CORES == 0
    g_loc = num_graphs // N_CORES

    counts = np.bincount(batch, minlength=num_graphs)
    starts = np.concatenate([[0], np.cumsum(counts)])
    # Size-balanced slot assignment: sort graphs by size (desc) and deal
    # round-robin, so slot j holds the 8 graphs ranked [8j, 8j+8) — the
    # per-slot max (which all 8 cores pad to) is then within a node or two
    # of every member, cutting pad DMA from ~5% to ~0.3%.
    order = np.argsort(-counts, kind="stable")
    assign = order.reshape(g_loc, N_CORES).T  # [core, slot] -> graph id
    slot_sizes = counts[assign].max(axis=0)
    slot_sizes = np.maximum(16, ((slot_sizes + 15) // 16) * 16).astype(np.int64)
    offs = np.concatenate([[0], np.cumsum(slot_sizes)])
    s_total = int(offs[-1])
    padw = int(slot_sizes.max())

    # pack: x_pad[c] = [H, s_total], graph assign[c, j] transposed at off_j
    x_pad = np.full((N_CORES, H, s_total), -np.inf, dtype=np.float32)
    for c in range(N_CORES):
        for j in range(g_loc):
            g = assign[c, j]
            s, e = starts[g], starts[g + 1]
            if e > s:
                x_pad[c, :, offs[j] : offs[j] + (e - s)] = x[s:e].T

    nc = _build_nc(slot_sizes, g_loc)

    eng = _eng_split(g_loc)
    cst = np.zeros((g_loc, 2), dtype=np.float32)
    for j in range(g_loc):
        if eng[j] == "a":
            cst[j, 0] = -127.5
            cst[j, 1] = float(padw - int(slot_sizes[j]))
        else:
            cst[j, 0] = 0.5
            cst[j, 1] = 0.0

    _install_ntff_hook_shim()
    from concourse.bass_utils import run_bass_kernel_spmd

    in_maps = [{"x": x_pad[c], "cst": cst} for c in range(N_CORES)]
    res = run_bass_kernel_spmd(nc, in_maps, core_ids=list(range(N_CORES)))
    _last_results = res

    graph_embedding = np.empty((num_graphs, H), dtype=np.float32)
    attention = np.empty(n_nodes, dtype=np.float32)
    for c in range(N_CORES):
        out = res.results[c]
        graph_embedding[assign[c]] = out["gmx"].T
        att_c = out["att"]
        for j in range(g_loc):
            g = assign[c, j]
            s, e = starts[g], starts[g + 1]
            if e > s:
                attention[s:e] = att_c[j, : e - s]
    return graph_embedding, attention


# revision 33
# speedup vs baseline: 1.0125x; 1.0125x over previous
"""Trainium2 Bass kernel for nn_MaxPooling (segment_max pooling + max-node
attention scores).

Strategy (per the segment-aligned sharding hint):
  - 1024 graphs are split 128-per-core across 8 NeuronCores; batch is sorted,
    so each graph's nodes are a contiguous row range of x.
  - Host packs each graph transposed: x_pad[core] is [128(hidden), S_total]
    with graph slot j occupying columns [off_j, off_j + S_j), padded with
    -inf to a per-slot size S_j (max over the 8 cores, 16-aligned).  With
    hidden on partitions:
      * segment_max  = one free-axis reduce_max per graph        (DVE)
      * per-dim match = tensor_scalar is_equal vs the gmax column (DVE/GPSIMD)
        or Sign(x - gmax) on the Scalar engine (ACT)  -> bf16 tile
      * per-node match count = ones^T @ match  (TensorE matmul, PSUM row per
        graph)  -> counts[graph, node]
      * scores = counts >= thr (thr 0.5 for eq rows, -127.5 for sign rows,
        PSUM pad memset to -1e9 so pad columns score 0)
      * totals = row-sum, attention = scores * (1/totals)
  - Outputs: gmax as [hidden, 128] per core and attention as [128, PADW]
    per core; host transposes/trims back into the full outputs.

Pad-node correctness: pad columns are -inf; for a non-empty graph the gmax
column is finite so is_equal(-inf, gmax)=0 and Sign(-inf - gmax)=-1, i.e.
pad nodes never score.  Empty graphs (cannot occur for these inputs) would
only corrupt their own discarded rows; their gmax stays -inf which matches
jax.ops.segment_max's identity.
"""

import os
import numpy as np

H = 128          # hidden dim == SBUF partitions
N_CORES = 8

# engine split for the per-graph match pass: slots [0, N_ACT) on ScalarE
# (Sign path), the rest on VectorE (is_equal).  GPSIMD is not used: its
# tensor-scalar is ~16 us per graph and it contends for the DVE SBUF port.
N_ACT = int(os.environ.get("K_NA", "128"))


def _eng_split(g_loc):
    return ["a" if j < N_ACT else "v" for j in range(g_loc)]

GROUP_W = 2560   # columns per input DMA (~1.25 MiB per dma_start)

_last_results = None  # BassKernelResults from the most recent run (for test.py)


def _apply_tile_patch():
    """This walrus build rejects instructions with >2 sync waits; the Tile
    kernel-tail drain accumulates one wait per live semaphore.  Split the
    drain's waits across single-wait SP nops."""
    import concourse.mybir as mybir
    import concourse.tile as tile_mod
    from concourse.vector_clock import ScopedClock

    if getattr(tile_mod.TileContext, "_ant_drain_patched", False):
        return

    def _drain_and_barrier(self, tick_clock, wait_clock):
        nc = self.nc
        drain_inst = nc.sync.drain()
        wait_clock.add_sem_waits(
            drain_inst.ins, ScopedClock({None: tick_clock.global_clock})
        )
        si = drain_inst.ins.sync_info
        waits = list(si.on_wait or []) if si is not None else []
        if len(waits) > 1:
            si.on_wait = waits[:1]
            for w in waits[1:]:
                nop = nc.sync.nop()
                nop.ins.sync_info = mybir.SyncInfo(on_wait=[w], on_update=[])
        nc.all_engine_barrier()
        assert self.sems is not None
        popped = nc._tile_sem_poison_stack.pop()
        assert popped is self._sem_poison
        nc.clear_and_free_semaphores(list(self.sems.allocated().values()))
        nc.all_engine_barrier()

    tile_mod.TileContext._drain_and_barrier = _drain_and_barrier
    tile_mod.TileContext._ant_drain_patched = True


def _split_excess_waits(nc, maxw=1):
    """Walrus here rejects instructions with more than ~1-2 sync waits.
    Hoist excess waits onto same-engine NoOps inserted just before the
    offending instruction (the engine blocks on the nop's wait first, so
    ordering semantics are identical)."""
    import bass_rust
    import concourse.mybir as mybir

    n = 0
    for f in nc.m.functions:
        for bb in f.blocks:
            out = []
            for inst in bb.instructions:
                si = inst.sync_info
                waits = list(si.on_wait or []) if si is not None else []
                if len(waits) > maxw and inst.engine is not None:
                    for i in range(0, len(waits) - maxw, maxw):
                        nop = bass_rust.InstNoOp(name=f"WSPLIT-{n}")
                        n += 1
                        nop.engine = inst.engine
                        nop.sync_info = mybir.SyncInfo(
                            on_wait=waits[i : i + maxw], on_update=[]
                        )
                        out.append(nop)
                    si.on_wait = waits[len(waits) - maxw :]
                out.append(inst)
            bb.instructions = out


def _install_ntff_hook_shim():
    """bass_utils hard-imports antenv.axon_hooks when trace=True under axon;
    this image's antenv lacks that module.  Provide it, wired to the
    libaxon_pjrt ctypes profiler from trn_agent_boot when available."""
    import sys
    import types

    try:
        import antenv.axon_hooks  # noqa: F401

        return
    except ImportError:
        pass
    try:
        import antenv
    except ImportError:
        return
    mod = types.ModuleType("antenv.axon_hooks")
    mod._hook = None
    mod.set_axon_ntff_profile_hook = lambda h: setattr(mod, "_hook", h)
    mod.get_axon_ntff_profile_hook = lambda: mod._hook
    sys.modules["antenv.axon_hooks"] = mod
    antenv.axon_hooks = mod
    try:
        from trn_agent_boot.trn_boot import _ntff_profile_via_ctypes

        hook = _ntff_profile_via_ctypes("/opt/axon/libaxon_pjrt.so")
        if hook is not None:
            mod._hook = hook
    except Exception:
        pass


def _build_nc(slot_sizes, g_loc):
    import concourse.bass as bass
    import concourse.mybir as mybir
    from concourse.tile import TileContext

    _apply_tile_patch()

    offs = np.concatenate([[0], np.cumsum(slot_sizes)])
    s_total = int(offs[-1])
    padw = int(max(slot_sizes))
    f32 = mybir.dt.float32
    bf16 = mybir.dt.bfloat16
    Alu = mybir.AluOpType
    X = mybir.AxisListType.X

    eng = _eng_split(g_loc)

    nc = bass.Bass()
    x_d = nc.dram_tensor("x", [H, s_total], f32, kind="ExternalInput")
    # per-graph-row constants: col 0 = score threshold (0.5 for is_equal
    # rows, -127.5 for Sign rows), col 1 = pad-column score correction
    # subtracted from the row total (padw - S_j for Sign rows, else 0).
    cst_d = nc.dram_tensor("cst", [g_loc, 2], f32, kind="ExternalInput")
    att_d = nc.dram_tensor("att", [g_loc, padw], f32, kind="ExternalOutput")
    gmx_d = nc.dram_tensor("gmx", [H, g_loc], f32, kind="ExternalOutput")

    # group consecutive slots into ~GROUP_W-column DMAs; the first few
    # slots go in single-graph DMAs so compute starts sooner
    groups = []
    cur = []
    cur_w = 0
    for j in range(g_loc):
        w = int(slot_sizes[j])
        if cur and cur_w + w > GROUP_W:
            groups.append(cur)
            cur, cur_w = [], 0
        cur.append(j)
        cur_w += w
    if cur:
        groups.append(cur)

    with TileContext(nc) as tc:
        with (
            tc.tile_pool(name="xg", bufs=8) as xp,
            tc.tile_pool(name="eq", bufs=8) as eqp,
            tc.tile_pool(name="small", bufs=1) as sp,
            tc.tile_pool(name="psum", bufs=1, space="PSUM") as pp,
        ):
            # sliding one-hot: win[:, 128-j : 256-j] is all-zero except an
            # all-ones column at free position j -> matmul lhsT that routes a
            # column-sum into PSUM partition row j (engine APs cannot start
            # at arbitrary partitions, so rows are selected via lhsT instead).
            # winn carries -1 so Sign rows accumulate (match-count - 128).
            win = sp.tile([H, 2 * g_loc], bf16)
            nc.gpsimd.memset(win[:], 0.0)
            nc.gpsimd.memset(win[:, g_loc : g_loc + 1], 1.0)
            winn = sp.tile([H, 2 * g_loc], bf16)
            nc.gpsimd.memset(winn[:], 0.0)
            nc.gpsimd.memset(winn[:, g_loc : g_loc + 1], -1.0)
            cst = sp.tile([g_loc, 2], f32)
            nc.sync.dma_start(out=cst[:], in_=cst_d[:])
            gsb = sp.tile([H, g_loc], f32)
            # counts[graph, node] accumulated in PSUM; pad columns stay 0
            counts = pp.tile([g_loc, padw], f32)
            nc.vector.memset(counts[:], 0.0)

            for grp in groups:
                g0, g1 = grp[0], grp[-1]
                base = int(offs[g0])
                gw = int(offs[g1 + 1]) - base
                xt = xp.tile([H, GROUP_W], f32, tag="xg")
                nc.sync.dma_start(out=xt[:, :gw], in_=x_d[:, base : base + gw])
                for j in grp:
                    lo = int(offs[j]) - base
                    sj = int(slot_sizes[j])
                    xg = xt[:, lo : lo + sj]
                    gcol = gsb[:, j : j + 1]
                    nc.vector.reduce_max(out=gcol, in_=xg, axis=X)
                    eqt = eqp.tile([H, padw], bf16, tag="eq")
                    eq = eqt[:, :sj]
                    if eng[j] == "v":
                        # eq in {0,1}; count row = match count m
                        nc.vector.tensor_single_scalar(
                            out=eq, in_=xg, scalar=gcol, op=Alu.is_equal
                        )
                        wt = win
                    else:
                        # sign(gmax - x) in {0 match, +1 not}; with the -1
                        # lhsT the count row = m - 128, thresholded -127.5
                        nc.scalar.activation(
                            out=eq,
                            in_=xg,
                            func=mybir.ActivationFunctionType.Sign,
                            bias=gcol,
                            scale=-1.0,
                        )
                        wt = winn
                    lhsT = wt[:, g_loc - j : 2 * g_loc - j]
                    for cs in range(0, sj, 512):
                        cw = min(512, sj - cs)
                        nc.tensor.matmul(
                            out=counts[:, cs : cs + cw],
                            lhsT=lhsT,
                            rhs=eqt[:, cs : cs + cw],
                            start=False,
                            stop=True,
                            skip_group_check=True,
                        )

            scores = sp.tile([g_loc, padw], f32)
            nc.vector.tensor_single_scalar(
                out=scores[:], in_=counts[:], scalar=cst[:, 0:1], op=Alu.is_ge
            )
            tot = sp.tile([g_loc, 1], f32)
            nc.vector.reduce_sum(out=tot[:], in_=scores[:], axis=X)
            # remove the pad columns' bogus unit scores on Sign rows
            nc.vector.tensor_sub(out=tot[:], in0=tot[:], in1=cst[:, 1:2])
            inv = sp.tile([g_loc, 1], f32)
            nc.vector.reciprocal(inv[:], tot[:])
            att = sp.tile([g_loc, padw], f32)
            nc.vector.tensor_scalar_mul(out=att[:], in0=scores[:], scalar1=inv[:, 0:1])
            nc.sync.dma_start(out=att_d[:], in_=att[:])
            nc.sync.dma_start(out=gmx_d[:], in_=gsb[:])

    _split_excess_waits(nc)
    return nc


def kernel(x, batch, num_graphs):
    global _last_results
    x = np.ascontiguousarray(np.asarray(x, dtype=np.float32))
    batch = np.asarray(batch).astype(np.int64)
    num_graphs = int(num_graphs)
    n_nodes, hidden = x.shape
    assert hidden == H and num_graphs % N_CORES == 0
    g_loc = num_graphs // N_CORES

    counts = np.bincount(batch, minlength=num_graphs)
    starts = np.concatenate([[0], np.cumsum(counts)])
    # Size-balanced slot assignment: sort graphs by size (desc) and deal
    # round-robin, so slot j holds the 8 graphs ranked [8j, 8j+8) — the
    # per-slot max (which all 8 cores pad to) is then within a node or two
    # of every member, cutting pad DMA from ~5% to ~0.3%.
    order = np.argsort(-counts, kind="stable")
    assign = order.reshape(g_loc, N_CORES).T  # [core, slot] -> graph id
    # interleave slot order (big/small alternating) to avoid a monotone
    # run of identical DMA shapes
    perm = np.empty(g_loc, dtype=np.int64)
    perm[0::2] = np.arange(g_loc // 2)
    perm[1::2] = np.arange(g_loc - 1, g_loc // 2 - 1, -1)
    assign = assign[:, perm]
    slot_sizes = counts[assign].max(axis=0)
    slot_sizes = np.maximum(16, ((slot_sizes + 15) // 16) * 16).astype(np.int64)
    offs = np.concatenate([[0], np.cumsum(slot_sizes)])
    s_total = int(offs[-1])
    padw = int(slot_sizes.max())

    # pack: x_pad[c] = [H, s_total], graph assign[c, j] transposed at off_j
    x_pad = np.full((N_CORES, H, s_total), -np.inf, dtype=np.float32)
    for c in range(N_CORES):
        for j in range(g_loc):
            g = assign[c, j]
            s, e = starts[g], starts[g + 1]
            if e > s:
                x_pad[c, :, offs[j] : offs[j] + (e - s)] = x[s:e].T

    nc = _build_nc(slot_sizes, g_loc)

    eng = _eng_split(g_loc)
    cst = np.zeros((g_loc, 2), dtype=np.float32)
    for j in range(g_loc):
        if eng[j] == "a":
            cst[j, 0] = -127.5
            cst[j, 1] = float(padw - int(slot_sizes[j]))
        else:
            cst[j, 0] = 0.5
            cst[j, 1] = 0.0

    _install_ntff_hook_shim()
    from concourse.bass_utils import run_bass_kernel_spmd

    in_maps = [{"x": x_pad[c], "cst": cst} for c in range(N_CORES)]
    res = run_bass_kernel_spmd(nc, in_maps, core_ids=list(range(N_CORES)))
    _last_results = res

    graph_embedding = np.empty((num_graphs, H), dtype=np.float32)
    attention = np.empty(n_nodes, dtype=np.float32)
    for c in range(N_CORES):
        out = res.results[c]
        graph_embedding[assign[c]] = out["gmx"].T
        att_c = out["att"]
        for j in range(g_loc):
            g = assign[c, j]
            s, e = starts[g], starts[g + 1]
            if e > s:
                attention[s:e] = att_c[j, : e - s]
    return graph_embedding, attention


# revision 34
# speedup vs baseline: 1.1655x; 1.1510x over previous
"""Trainium2 Bass kernel for nn_MaxPooling (segment_max pooling + max-node
attention scores).

Strategy (per the segment-aligned sharding hint):
  - 1024 graphs are split 128-per-core across 8 NeuronCores; batch is sorted,
    so each graph's nodes are a contiguous row range of x.
  - Host packs each graph transposed: x_pad[core] is [128(hidden), S_total]
    with graph slot j occupying columns [off_j, off_j + S_j), padded with
    -inf to a per-slot size S_j (max over the 8 cores, 16-aligned).  With
    hidden on partitions:
      * segment_max  = one free-axis reduce_max per graph        (DVE)
      * per-dim match = tensor_scalar is_equal vs the gmax column (DVE/GPSIMD)
        or Sign(x - gmax) on the Scalar engine (ACT)  -> bf16 tile
      * per-node match count = ones^T @ match  (TensorE matmul, PSUM row per
        graph)  -> counts[graph, node]
      * scores = counts >= thr (thr 0.5 for eq rows, -127.5 for sign rows,
        PSUM pad memset to -1e9 so pad columns score 0)
      * totals = row-sum, attention = scores * (1/totals)
  - Outputs: gmax as [hidden, 128] per core and attention as [128, PADW]
    per core; host transposes/trims back into the full outputs.

Pad-node correctness: pad columns are -inf; for a non-empty graph the gmax
column is finite so is_equal(-inf, gmax)=0 and Sign(-inf - gmax)=-1, i.e.
pad nodes never score.  Empty graphs (cannot occur for these inputs) would
only corrupt their own discarded rows; their gmax stays -inf which matches
jax.ops.segment_max's identity.
"""

import os
import numpy as np

H = 128          # hidden dim == SBUF partitions
N_CORES = 8

# engine split for the per-graph match pass: slots [0, N_ACT) on ScalarE
# (Sign path), the rest on VectorE (is_equal).  GPSIMD is not used: its
# tensor-scalar is ~16 us per graph and it contends for the DVE SBUF port.
N_ACT = int(os.environ.get("K_NA", "128"))


def _eng_split(g_loc):
    return ["a" if j < N_ACT else "v" for j in range(g_loc)]

GROUP_W = 2560   # columns per input DMA (~1.25 MiB per dma_start)

_last_results = None  # BassKernelResults from the most recent run (for test.py)


def _apply_tile_patch():
    """This walrus build rejects instructions with >2 sync waits; the Tile
    kernel-tail drain accumulates one wait per live semaphore.  Split the
    drain's waits across single-wait SP nops."""
    import concourse.mybir as mybir
    import concourse.tile as tile_mod
    from concourse.vector_clock import ScopedClock

    if getattr(tile_mod.TileContext, "_ant_drain_patched", False):
        return

    def _drain_and_barrier(self, tick_clock, wait_clock):
        nc = self.nc
        drain_inst = nc.sync.drain()
        wait_clock.add_sem_waits(
            drain_inst.ins, ScopedClock({None: tick_clock.global_clock})
        )
        si = drain_inst.ins.sync_info
        waits = list(si.on_wait or []) if si is not None else []
        if len(waits) > 1:
            si.on_wait = waits[:1]
            for w in waits[1:]:
                nop = nc.sync.nop()
                nop.ins.sync_info = mybir.SyncInfo(on_wait=[w], on_update=[])
        nc.all_engine_barrier()
        assert self.sems is not None
        popped = nc._tile_sem_poison_stack.pop()
        assert popped is self._sem_poison
        nc.clear_and_free_semaphores(list(self.sems.allocated().values()))
        nc.all_engine_barrier()

    tile_mod.TileContext._drain_and_barrier = _drain_and_barrier
    tile_mod.TileContext._ant_drain_patched = True


def _split_excess_waits(nc, maxw=1):
    """Walrus here rejects instructions with more than ~1-2 sync waits.
    Hoist excess waits onto same-engine NoOps inserted just before the
    offending instruction (the engine blocks on the nop's wait first, so
    ordering semantics are identical)."""
    import bass_rust
    import concourse.mybir as mybir

    n = 0
    for f in nc.m.functions:
        for bb in f.blocks:
            out = []
            for inst in bb.instructions:
                si = inst.sync_info
                waits = list(si.on_wait or []) if si is not None else []
                if len(waits) > maxw and inst.engine is not None:
                    for i in range(0, len(waits) - maxw, maxw):
                        nop = bass_rust.InstNoOp(name=f"WSPLIT-{n}")
                        n += 1
                        nop.engine = inst.engine
                        nop.sync_info = mybir.SyncInfo(
                            on_wait=waits[i : i + maxw], on_update=[]
                        )
                        out.append(nop)
                    si.on_wait = waits[len(waits) - maxw :]
                out.append(inst)
            bb.instructions = out


def _install_ntff_hook_shim():
    """bass_utils hard-imports antenv.axon_hooks when trace=True under axon;
    this image's antenv lacks that module.  Provide it, wired to the
    libaxon_pjrt ctypes profiler from trn_agent_boot when available."""
    import sys
    import types

    try:
        import antenv.axon_hooks  # noqa: F401

        return
    except ImportError:
        pass
    try:
        import antenv
    except ImportError:
        return
    mod = types.ModuleType("antenv.axon_hooks")
    mod._hook = None
    mod.set_axon_ntff_profile_hook = lambda h: setattr(mod, "_hook", h)
    mod.get_axon_ntff_profile_hook = lambda: mod._hook
    sys.modules["antenv.axon_hooks"] = mod
    antenv.axon_hooks = mod
    try:
        from trn_agent_boot.trn_boot import _ntff_profile_via_ctypes

        hook = _ntff_profile_via_ctypes("/opt/axon/libaxon_pjrt.so")
        if hook is not None:
            mod._hook = hook
    except Exception:
        pass


def _build_nc(slot_sizes, g_loc):
    import concourse.bass as bass
    import concourse.mybir as mybir
    from concourse.tile import TileContext

    _apply_tile_patch()

    offs = np.concatenate([[0], np.cumsum(slot_sizes)])
    s_total = int(offs[-1])
    padw = int(max(slot_sizes))
    f32 = mybir.dt.float32
    bf16 = mybir.dt.bfloat16
    Alu = mybir.AluOpType
    X = mybir.AxisListType.X

    eng = _eng_split(g_loc)

    nc = bass.Bass()
    x_d = nc.dram_tensor("x", [H, s_total], f32, kind="ExternalInput")
    # per-graph-row constants: col 0 = score threshold (0.5 for is_equal
    # rows, -127.5 for Sign rows), col 1 = pad-column score correction
    # subtracted from the row total (padw - S_j for Sign rows, else 0).
    cst_d = nc.dram_tensor("cst", [g_loc, 2], f32, kind="ExternalInput")
    att_d = nc.dram_tensor("att", [g_loc, padw], f32, kind="ExternalOutput")
    gmx_d = nc.dram_tensor("gmx", [H, g_loc], f32, kind="ExternalOutput")

    # group consecutive slots into ~GROUP_W-column DMAs; the first few
    # slots go in single-graph DMAs so compute starts sooner
    groups = []
    cur = []
    cur_w = 0
    for j in range(g_loc):
        w = int(slot_sizes[j])
        if cur and cur_w + w > GROUP_W:
            groups.append(cur)
            cur, cur_w = [], 0
        cur.append(j)
        cur_w += w
    if cur:
        groups.append(cur)

    with TileContext(nc) as tc:
        with (
            tc.tile_pool(name="xg", bufs=8) as xp,
            tc.tile_pool(name="eq", bufs=8) as eqp,
            tc.tile_pool(name="small", bufs=1) as sp,
            tc.tile_pool(name="psum", bufs=1, space="PSUM") as pp,
        ):
            # sliding one-hot: win[:, 128-j : 256-j] is all-zero except an
            # all-ones column at free position j -> matmul lhsT that routes a
            # column-sum into PSUM partition row j (engine APs cannot start
            # at arbitrary partitions, so rows are selected via lhsT instead).
            # winn carries -1 so Sign rows accumulate (match-count - 128).
            win = sp.tile([H, 2 * g_loc], bf16)
            nc.gpsimd.memset(win[:], 0.0)
            nc.gpsimd.memset(win[:, g_loc : g_loc + 1], 1.0)
            winn = sp.tile([H, 2 * g_loc], bf16)
            nc.gpsimd.memset(winn[:], 0.0)
            nc.gpsimd.memset(winn[:, g_loc : g_loc + 1], -1.0)
            cst = sp.tile([g_loc, 2], f32)
            nc.sync.dma_start(out=cst[:], in_=cst_d[:])
            gsb = sp.tile([H, g_loc], f32)
            # counts[graph, node] accumulated in PSUM; pad columns stay 0
            counts = pp.tile([g_loc, padw], f32)
            nc.vector.memset(counts[:], 0.0)

            for grp in groups:
                g0, g1 = grp[0], grp[-1]
                base = int(offs[g0])
                gw = int(offs[g1 + 1]) - base
                xt = xp.tile([H, GROUP_W], f32, tag="xg")
                nc.sync.dma_start(out=xt[:, :gw], in_=x_d[:, base : base + gw])
                for j in grp:
                    lo = int(offs[j]) - base
                    sj = int(slot_sizes[j])
                    xg = xt[:, lo : lo + sj]
                    gcol = gsb[:, j : j + 1]
                    nc.vector.reduce_max(out=gcol, in_=xg, axis=X)
                    eqt = eqp.tile([H, padw], bf16, tag="eq")
                    eq = eqt[:, :sj]
                    if eng[j] == "v":
                        # eq in {0,1}; count row = match count m
                        nc.vector.tensor_single_scalar(
                            out=eq, in_=xg, scalar=gcol, op=Alu.is_equal
                        )
                        wt = win
                    else:
                        # sign(gmax - x) in {0 match, +1 not}; with the -1
                        # lhsT the count row = m - 128, thresholded -127.5
                        nc.scalar.activation(
                            out=eq,
                            in_=xg,
                            func=mybir.ActivationFunctionType.Sign,
                            bias=gcol,
                            scale=-1.0,
                        )
                        wt = winn
                    lhsT = wt[:, g_loc - j : 2 * g_loc - j]
                    for cs in range(0, sj, 512):
                        cw = min(512, sj - cs)
                        nc.tensor.matmul(
                            out=counts[:, cs : cs + cw],
                            lhsT=lhsT,
                            rhs=eqt[:, cs : cs + cw],
                            start=False,
                            stop=True,
                            skip_group_check=True,
                        )

            scores = sp.tile([g_loc, padw], f32)
            nc.vector.tensor_single_scalar(
                out=scores[:], in_=counts[:], scalar=cst[:, 0:1], op=Alu.is_ge
            )
            tot = sp.tile([g_loc, 1], f32)
            nc.vector.reduce_sum(out=tot[:], in_=scores[:], axis=X)
            # remove the pad columns' bogus unit scores on Sign rows
            nc.vector.tensor_sub(out=tot[:], in0=tot[:], in1=cst[:, 1:2])
            inv = sp.tile([g_loc, 1], f32)
            nc.vector.reciprocal(inv[:], tot[:])
            att = sp.tile([g_loc, padw], f32)
            nc.vector.tensor_scalar_mul(out=att[:], in0=scores[:], scalar1=inv[:, 0:1])
            nc.sync.dma_start(out=att_d[:], in_=att[:])
            nc.sync.dma_start(out=gmx_d[:], in_=gsb[:])

    _split_excess_waits(nc)
    return nc


def kernel(x, batch, num_graphs):
    global _last_results
    x = np.ascontiguousarray(np.asarray(x, dtype=np.float32))
    batch = np.asarray(batch).astype(np.int64)
    num_graphs = int(num_graphs)
    n_nodes, hidden = x.shape
    assert hidden == H and num_graphs % N_CORES == 0
    g_loc = num_graphs // N_CORES

    counts = np.bincount(batch, minlength=num_graphs)
    starts = np.concatenate([[0], np.cumsum(counts)])
    # Size-balanced slot assignment: sort graphs by size (desc) and deal
    # round-robin, so slot j holds the 8 graphs ranked [8j, 8j+8) — the
    # per-slot max (which all 8 cores pad to) is then within a node or two
    # of every member, cutting pad DMA from ~5% to ~0.3%.
    order = np.argsort(-counts, kind="stable")
    assign = order.reshape(g_loc, N_CORES).T  # [core, slot] -> graph id
    # interleave slot order (big/small alternating) to avoid a monotone
    # run of identical DMA shapes
    perm = np.empty(g_loc, dtype=np.int64)
    perm[0::2] = np.arange(g_loc // 2)
    perm[1::2] = np.arange(g_loc - 1, g_loc // 2 - 1, -1)
    assign = assign[:, perm]
    slot_sizes = counts[assign].max(axis=0)
    slot_sizes = np.maximum(64, ((slot_sizes + 63) // 64) * 64).astype(np.int64)
    offs = np.concatenate([[0], np.cumsum(slot_sizes)])
    s_total = int(offs[-1])
    padw = int(slot_sizes.max())

    # pack: x_pad[c] = [H, s_total], graph assign[c, j] transposed at off_j
    x_pad = np.full((N_CORES, H, s_total), -np.inf, dtype=np.float32)
    for c in range(N_CORES):
        for j in range(g_loc):
            g = assign[c, j]
            s, e = starts[g], starts[g + 1]
            if e > s:
                x_pad[c, :, offs[j] : offs[j] + (e - s)] = x[s:e].T

    nc = _build_nc(slot_sizes, g_loc)

    eng = _eng_split(g_loc)
    cst = np.zeros((g_loc, 2), dtype=np.float32)
    for j in range(g_loc):
        if eng[j] == "a":
            cst[j, 0] = -127.5
            cst[j, 1] = float(padw - int(slot_sizes[j]))
        else:
            cst[j, 0] = 0.5
            cst[j, 1] = 0.0

    _install_ntff_hook_shim()
    from concourse.bass_utils import run_bass_kernel_spmd

    in_maps = [{"x": x_pad[c], "cst": cst} for c in range(N_CORES)]
    res = run_bass_kernel_spmd(nc, in_maps, core_ids=list(range(N_CORES)))
    _last_results = res

    graph_embedding = np.empty((num_graphs, H), dtype=np.float32)
    attention = np.empty(n_nodes, dtype=np.float32)
    for c in range(N_CORES):
        out = res.results[c]
        graph_embedding[assign[c]] = out["gmx"].T
        att_c = out["att"]
        for j in range(g_loc):
            g = assign[c, j]
            s, e = starts[g], starts[g + 1]
            if e > s:
                attention[s:e] = att_c[j, : e - s]
    return graph_embedding, attention


# revision 35
# speedup vs baseline: 1.1729x; 1.0064x over previous
"""Trainium2 Bass kernel for nn_MaxPooling (segment_max pooling + max-node
attention scores).

Strategy (per the segment-aligned sharding hint):
  - 1024 graphs are split 128-per-core across 8 NeuronCores; batch is sorted,
    so each graph's nodes are a contiguous row range of x.
  - Host packs each graph transposed: x_pad[core] is [128(hidden), S_total]
    with graph slot j occupying columns [off_j, off_j + S_j), padded with
    -inf to a per-slot size S_j (max over the 8 cores, 16-aligned).  With
    hidden on partitions:
      * segment_max  = one free-axis reduce_max per graph        (DVE)
      * per-dim match = tensor_scalar is_equal vs the gmax column (DVE/GPSIMD)
        or Sign(x - gmax) on the Scalar engine (ACT)  -> bf16 tile
      * per-node match count = ones^T @ match  (TensorE matmul, PSUM row per
        graph)  -> counts[graph, node]
      * scores = counts >= thr (thr 0.5 for eq rows, -127.5 for sign rows,
        PSUM pad memset to -1e9 so pad columns score 0)
      * totals = row-sum, attention = scores * (1/totals)
  - Outputs: gmax as [hidden, 128] per core and attention as [128, PADW]
    per core; host transposes/trims back into the full outputs.

Pad-node correctness: pad columns are -inf; for a non-empty graph the gmax
column is finite so is_equal(-inf, gmax)=0 and Sign(-inf - gmax)=-1, i.e.
pad nodes never score.  Empty graphs (cannot occur for these inputs) would
only corrupt their own discarded rows; their gmax stays -inf which matches
jax.ops.segment_max's identity.
"""

import os
import numpy as np

H = 128          # hidden dim == SBUF partitions
N_CORES = 8

# engine split for the per-graph match pass: slots [0, N_ACT) on ScalarE
# (Sign path), the rest on VectorE (is_equal).  GPSIMD is not used: its
# tensor-scalar is ~16 us per graph and it contends for the DVE SBUF port.
N_ACT = int(os.environ.get("K_NA", "128"))


def _eng_split(g_loc):
    return ["a" if j < N_ACT else "v" for j in range(g_loc)]

GROUP_W = 3840   # columns per input DMA (~1.25 MiB per dma_start)

_last_results = None  # BassKernelResults from the most recent run (for test.py)


def _apply_tile_patch():
    """This walrus build rejects instructions with >2 sync waits; the Tile
    kernel-tail drain accumulates one wait per live semaphore.  Split the
    drain's waits across single-wait SP nops."""
    import concourse.mybir as mybir
    import concourse.tile as tile_mod
    from concourse.vector_clock import ScopedClock

    if getattr(tile_mod.TileContext, "_ant_drain_patched", False):
        return

    def _drain_and_barrier(self, tick_clock, wait_clock):
        nc = self.nc
        drain_inst = nc.sync.drain()
        wait_clock.add_sem_waits(
            drain_inst.ins, ScopedClock({None: tick_clock.global_clock})
        )
        si = drain_inst.ins.sync_info
        waits = list(si.on_wait or []) if si is not None else []
        if len(waits) > 1:
            si.on_wait = waits[:1]
            for w in waits[1:]:
                nop = nc.sync.nop()
                nop.ins.sync_info = mybir.SyncInfo(on_wait=[w], on_update=[])
        nc.all_engine_barrier()
        assert self.sems is not None
        popped = nc._tile_sem_poison_stack.pop()
        assert popped is self._sem_poison
        nc.clear_and_free_semaphores(list(self.sems.allocated().values()))
        nc.all_engine_barrier()

    tile_mod.TileContext._drain_and_barrier = _drain_and_barrier
    tile_mod.TileContext._ant_drain_patched = True


def _split_excess_waits(nc, maxw=1):
    """Walrus here rejects instructions with more than ~1-2 sync waits.
    Hoist excess waits onto same-engine NoOps inserted just before the
    offending instruction (the engine blocks on the nop's wait first, so
    ordering semantics are identical)."""
    import bass_rust
    import concourse.mybir as mybir

    n = 0
    for f in nc.m.functions:
        for bb in f.blocks:
            out = []
            for inst in bb.instructions:
                si = inst.sync_info
                waits = list(si.on_wait or []) if si is not None else []
                if len(waits) > maxw and inst.engine is not None:
                    for i in range(0, len(waits) - maxw, maxw):
                        nop = bass_rust.InstNoOp(name=f"WSPLIT-{n}")
                        n += 1
                        nop.engine = inst.engine
                        nop.sync_info = mybir.SyncInfo(
                            on_wait=waits[i : i + maxw], on_update=[]
                        )
                        out.append(nop)
                    si.on_wait = waits[len(waits) - maxw :]
                out.append(inst)
            bb.instructions = out


def _install_ntff_hook_shim():
    """bass_utils hard-imports antenv.axon_hooks when trace=True under axon;
    this image's antenv lacks that module.  Provide it, wired to the
    libaxon_pjrt ctypes profiler from trn_agent_boot when available."""
    import sys
    import types

    try:
        import antenv.axon_hooks  # noqa: F401

        return
    except ImportError:
        pass
    try:
        import antenv
    except ImportError:
        return
    mod = types.ModuleType("antenv.axon_hooks")
    mod._hook = None
    mod.set_axon_ntff_profile_hook = lambda h: setattr(mod, "_hook", h)
    mod.get_axon_ntff_profile_hook = lambda: mod._hook
    sys.modules["antenv.axon_hooks"] = mod
    antenv.axon_hooks = mod
    try:
        from trn_agent_boot.trn_boot import _ntff_profile_via_ctypes

        hook = _ntff_profile_via_ctypes("/opt/axon/libaxon_pjrt.so")
        if hook is not None:
            mod._hook = hook
    except Exception:
        pass


def _build_nc(slot_sizes, g_loc):
    import concourse.bass as bass
    import concourse.mybir as mybir
    from concourse.tile import TileContext

    _apply_tile_patch()

    offs = np.concatenate([[0], np.cumsum(slot_sizes)])
    s_total = int(offs[-1])
    padw = int(max(slot_sizes))
    f32 = mybir.dt.float32
    bf16 = mybir.dt.bfloat16
    Alu = mybir.AluOpType
    X = mybir.AxisListType.X

    eng = _eng_split(g_loc)

    nc = bass.Bass()
    x_d = nc.dram_tensor("x", [H, s_total], f32, kind="ExternalInput")
    # per-graph-row constants: col 0 = score threshold (0.5 for is_equal
    # rows, -127.5 for Sign rows), col 1 = pad-column score correction
    # subtracted from the row total (padw - S_j for Sign rows, else 0).
    cst_d = nc.dram_tensor("cst", [g_loc, 2], f32, kind="ExternalInput")
    att_d = nc.dram_tensor("att", [g_loc, padw], f32, kind="ExternalOutput")
    gmx_d = nc.dram_tensor("gmx", [H, g_loc], f32, kind="ExternalOutput")

    # group consecutive slots into ~GROUP_W-column DMAs; the first few
    # slots go in single-graph DMAs so compute starts sooner
    groups = []
    cur = []
    cur_w = 0
    for j in range(g_loc):
        w = int(slot_sizes[j])
        if cur and cur_w + w > GROUP_W:
            groups.append(cur)
            cur, cur_w = [], 0
        cur.append(j)
        cur_w += w
    if cur:
        groups.append(cur)

    with TileContext(nc) as tc:
        with (
            tc.tile_pool(name="xg", bufs=8) as xp,
            tc.tile_pool(name="eq", bufs=8) as eqp,
            tc.tile_pool(name="small", bufs=1) as sp,
            tc.tile_pool(name="psum", bufs=1, space="PSUM") as pp,
        ):
            # sliding one-hot: win[:, 128-j : 256-j] is all-zero except an
            # all-ones column at free position j -> matmul lhsT that routes a
            # column-sum into PSUM partition row j (engine APs cannot start
            # at arbitrary partitions, so rows are selected via lhsT instead).
            # winn carries -1 so Sign rows accumulate (match-count - 128).
            win = sp.tile([H, 2 * g_loc], bf16)
            nc.gpsimd.memset(win[:], 0.0)
            nc.gpsimd.memset(win[:, g_loc : g_loc + 1], 1.0)
            winn = sp.tile([H, 2 * g_loc], bf16)
            nc.gpsimd.memset(winn[:], 0.0)
            nc.gpsimd.memset(winn[:, g_loc : g_loc + 1], -1.0)
            cst = sp.tile([g_loc, 2], f32)
            nc.sync.dma_start(out=cst[:], in_=cst_d[:])
            gsb = sp.tile([H, g_loc], f32)
            # counts[graph, node] accumulated in PSUM; pad columns stay 0
            counts = pp.tile([g_loc, padw], f32)
            nc.vector.memset(counts[:], 0.0)

            for grp in groups:
                g0, g1 = grp[0], grp[-1]
                base = int(offs[g0])
                gw = int(offs[g1 + 1]) - base
                xt = xp.tile([H, GROUP_W], f32, tag="xg")
                nc.sync.dma_start(out=xt[:, :gw], in_=x_d[:, base : base + gw])
                for j in grp:
                    lo = int(offs[j]) - base
                    sj = int(slot_sizes[j])
                    xg = xt[:, lo : lo + sj]
                    gcol = gsb[:, j : j + 1]
                    nc.vector.reduce_max(out=gcol, in_=xg, axis=X)
                    eqt = eqp.tile([H, padw], bf16, tag="eq")
                    eq = eqt[:, :sj]
                    if eng[j] == "v":
                        # eq in {0,1}; count row = match count m
                        nc.vector.tensor_single_scalar(
                            out=eq, in_=xg, scalar=gcol, op=Alu.is_equal
                        )
                        wt = win
                    else:
                        # sign(gmax - x) in {0 match, +1 not}; with the -1
                        # lhsT the count row = m - 128, thresholded -127.5
                        nc.scalar.activation(
                            out=eq,
                            in_=xg,
                            func=mybir.ActivationFunctionType.Sign,
                            bias=gcol,
                            scale=-1.0,
                        )
                        wt = winn
                    lhsT = wt[:, g_loc - j : 2 * g_loc - j]
                    for cs in range(0, sj, 512):
                        cw = min(512, sj - cs)
                        nc.tensor.matmul(
                            out=counts[:, cs : cs + cw],
                            lhsT=lhsT,
                            rhs=eqt[:, cs : cs + cw],
                            start=False,
                            stop=True,
                            skip_group_check=True,
                        )

            scores = sp.tile([g_loc, padw], f32)
            nc.vector.tensor_single_scalar(
                out=scores[:], in_=counts[:], scalar=cst[:, 0:1], op=Alu.is_ge
            )
            tot = sp.tile([g_loc, 1], f32)
            nc.vector.reduce_sum(out=tot[:], in_=scores[:], axis=X)
            # remove the pad columns' bogus unit scores on Sign rows
            nc.vector.tensor_sub(out=tot[:], in0=tot[:], in1=cst[:, 1:2])
            inv = sp.tile([g_loc, 1], f32)
            nc.vector.reciprocal(inv[:], tot[:])
            att = sp.tile([g_loc, padw], f32)
            nc.vector.tensor_scalar_mul(out=att[:], in0=scores[:], scalar1=inv[:, 0:1])
            nc.sync.dma_start(out=att_d[:], in_=att[:])
            nc.sync.dma_start(out=gmx_d[:], in_=gsb[:])

    _split_excess_waits(nc)
    return nc


def kernel(x, batch, num_graphs):
    global _last_results
    x = np.ascontiguousarray(np.asarray(x, dtype=np.float32))
    batch = np.asarray(batch).astype(np.int64)
    num_graphs = int(num_graphs)
    n_nodes, hidden = x.shape
    assert hidden == H and num_graphs % N_CORES == 0
    g_loc = num_graphs // N_CORES

    counts = np.bincount(batch, minlength=num_graphs)
    starts = np.concatenate([[0], np.cumsum(counts)])
    # Size-balanced slot assignment: sort graphs by size (desc) and deal
    # round-robin, so slot j holds the 8 graphs ranked [8j, 8j+8) — the
    # per-slot max (which all 8 cores pad to) is then within a node or two
    # of every member, cutting pad DMA from ~5% to ~0.3%.
    order = np.argsort(-counts, kind="stable")
    assign = order.reshape(g_loc, N_CORES).T  # [core, slot] -> graph id
    # interleave slot order (big/small alternating) to avoid a monotone
    # run of identical DMA shapes
    perm = np.empty(g_loc, dtype=np.int64)
    perm[0::2] = np.arange(g_loc // 2)
    perm[1::2] = np.arange(g_loc - 1, g_loc // 2 - 1, -1)
    assign = assign[:, perm]
    slot_sizes = counts[assign].max(axis=0)
    slot_sizes = np.maximum(64, ((slot_sizes + 63) // 64) * 64).astype(np.int64)
    offs = np.concatenate([[0], np.cumsum(slot_sizes)])
    s_total = int(offs[-1])
    padw = int(slot_sizes.max())

    # pack: x_pad[c] = [H, s_total], graph assign[c, j] transposed at off_j
    x_pad = np.full((N_CORES, H, s_total), -np.inf, dtype=np.float32)
    for c in range(N_CORES):
        for j in range(g_loc):
            g = assign[c, j]
            s, e = starts[g], starts[g + 1]
            if e > s:
                x_pad[c, :, offs[j] : offs[j] + (e - s)] = x[s:e].T

    nc = _build_nc(slot_sizes, g_loc)

    eng = _eng_split(g_loc)
    cst = np.zeros((g_loc, 2), dtype=np.float32)
    for j in range(g_loc):
        if eng[j] == "a":
            cst[j, 0] = -127.5
            cst[j, 1] = float(padw - int(slot_sizes[j]))
        else:
            cst[j, 0] = 0.5
            cst[j, 1] = 0.0

    _install_ntff_hook_shim()
    from concourse.bass_utils import run_bass_kernel_spmd

    in_maps = [{"x": x_pad[c], "cst": cst} for c in range(N_CORES)]
    res = run_bass_kernel_spmd(nc, in_maps, core_ids=list(range(N_CORES)))
    _last_results = res

    graph_embedding = np.empty((num_graphs, H), dtype=np.float32)
    attention = np.empty(n_nodes, dtype=np.float32)
    for c in range(N_CORES):
        out = res.results[c]
        graph_embedding[assign[c]] = out["gmx"].T
        att_c = out["att"]
        for j in range(g_loc):
            g = assign[c, j]
            s, e = starts[g], starts[g + 1]
            if e > s:
                attention[s:e] = att_c[j, : e - s]
    return graph_embedding, attention
